# revision 14
# baseline (speedup 1.0000x reference)
"""Multi-head causal attention with RoPE on 8 Trainium2 NeuronCores.

Sharding: tensor-parallel over heads x data-parallel over batch.
Core c handles batch b = c//4 and heads [4*(c%4), 4*(c%4)+4) (Hl=256 of Hd=1024).
Each core computes q/k/v projections for its head slice (column-split Wq/Wk/Wv),
RoPE, causal softmax attention, and a partial output projection (row-split Wo).
The host sums the 4 partial fp16 outputs per batch (the "all-reduce").

Fully fused single pipeline (v2): projections, rope, attention
(scores/exp/mask/pv), normalization and the output projection are emitted as
one interleaved instruction stream so the tensor engine never idles long
enough to re-throttle (HAM) and the ACT exp stream starts within a few us.

Device layouts (per core, S=2048, E=1024, Hl=256, D=64):
  xT   [E, S]    x transposed (host-side) so E rides the partition dim
  qT/kT slabs [128, S] x2: partitions = 2 heads x 64 dims, free = seq
  v    16 tiles [128, 512]: partitions = seq chunk, free = 4 heads x
       (64 dims + 64 ones cols) -> PV matmuls have M=128 and produce
       O on psum rows 0-63 and Z replicated on rows 64-127, so softmax
       normalization is two cross-partition-base reciprocal_approx_fast +
       two tensor_muls (no DRAM broadcast roundtrip).
  scores computed transposed (keys on partitions), exp on ACT (scale=0.125),
  causal masking via gpsimd affine_select on the diagonal chunks.

All matmul operands are fp16 (fp32 PSUM accumulation). A short burst of
dummy warm-up matmuls runs while the first input DMAs stream so the PE HAM
clock-gate is already released when real work arrives.
"""
import sys

sys.path.insert(0, "/opt/trn_rl_repo")
import numpy as np  # noqa: E402

N_HEADS = 16
B, S, E, HD = 2, 2048, 1024, 1024
D = HD // N_HEADS  # 64
HPC = 4            # heads per core
HL = HPC * D       # 256
NCORES = 8
ROPE_BASE = 10000.0

_built = None


def _build_nc():
    import concourse.bass as bass
    import concourse.tile as tile
    from concourse import bacc, mybir

    F32 = mybir.dt.float32
    F16 = mybir.dt.float16
    Exp = mybir.ActivationFunctionType.Exp
    is_ge = mybir.AluOpType.is_ge
    ts = bass.ts

    nc = bacc.Bacc("TRN2", target_bir_lowering=False, debug=False)
    xT_d = nc.dram_tensor("xT", [E, S], F16, kind="ExternalInput").ap()
    wq_d = nc.dram_tensor("wq", [E, HL], F16, kind="ExternalInput").ap()
    wk_d = nc.dram_tensor("wk", [E, HL], F16, kind="ExternalInput").ap()
    wv_d = nc.dram_tensor("wv", [E, HL], F16, kind="ExternalInput").ap()
    wo_d = nc.dram_tensor("wo", [HL, E], F16, kind="ExternalInput").ap()
    cos_d = nc.dram_tensor("cosx", [128, S], F16, kind="ExternalInput").ap()
    sin_d = nc.dram_tensor("sinx", [128, S], F16, kind="ExternalInput").ap()
    out_d = nc.dram_tensor("out", [S, E], F16, kind="ExternalOutput").ap()
    wrm_d = nc.dram_tensor("wrm", [1, 8], F32).ap()  # warmup sink

    ECH = E // 128   # 8 e-chunks
    SCH = S // 128   # 16 seq chunks
    SB = S // 512    # 4 seq blocks
    swap_mask = []
    for i in range(16):
        swap_mask += [2 * i + 1, 2 * i]

    with tile.TileContext(nc) as tc:
        with (
            tc.tile_pool(name="persist", bufs=1) as pp,
            tc.tile_pool(name="bswp", bufs=2) as bswp,
            tc.tile_pool(name="cexp", bufs=6) as cexp,
            tc.tile_pool(name="crb", bufs=2) as crb,
            tc.tile_pool(name="evict", bufs=4) as ev,
            tc.tile_pool(name="mm", bufs=2, space="PSUM") as mmp,
            tc.tile_pool(name="csc", bufs=2, space="PSUM") as csc,
            tc.tile_pool(name="cpv", bufs=1, space="PSUM") as cpv,
        ):
            # ---------------- persistent tiles ----------------
            qT = [pp.tile([128, S], F16, tag=f"qT{c}", name=f"qT{c}") for c in range(2)]
            kT = [pp.tile([128, S], F16, tag=f"kT{c}", name=f"kT{c}") for c in range(2)]
            vt = [pp.tile([128, HPC * 2 * D], F16, tag=f"v{t}", name=f"v{t}")
                  for t in range(SCH)]
            oT = [pp.tile([128, S], F16, tag=f"oT{c}", name=f"oT{c}") for c in range(2)]
            cosx = pp.tile([128, S], F16, tag="cosx", name="cosx")
            sinx = pp.tile([128, S], F16, tag="sinx", name="sinx")
            wo_t = pp.tile([128, 2, E], F16, tag="wo", name="wo")
            wq_t = pp.tile([128, ECH, HL], F16, tag="wq", name="wq")
            wk_t = pp.tile([128, ECH, HL], F16, tag="wk", name="wk")
            wv_t = pp.tile([128, ECH, HL], F16, tag="wv", name="wv")
            xt = [pp.tile([128, S], F16, tag=f"x{e}", name=f"x{e}")
                  for e in range(ECH)]
            wrm = pp.tile([128, 512], F16, tag="wrm", name="wrm")
            wrs = pp.tile([1, 8], F32, tag="wrs", name="wrs")

            # ---------------- PE warm-up (HAM release) ----------------
            # first gpsimd op so the warm-up matmuls start within ~1us
            nc.gpsimd.memset(wrm[:], 0.0)
            wps = mmp.tile([128, 512], F32, tag="mm", name="wps")
            for _ in range(12):
                nc.tensor.matmul(
                    wps[:], wrm[:, 0:128], wrm[:], start=True, stop=True
                )
            nc.vector.tensor_copy(out=wrs[:], in_=wps[0:1, 0:8])

            # ---------------- input DMAs ----------------
            # j0-critical data first, spread across four engine DMA queues
            # so the first projection can start ~4-5us in.
            def wdma(eng, w_t_, w_d_, sl=None):
                if sl is None:
                    eng.dma_start(
                        out=w_t_[:],
                        in_=w_d_.rearrange("(c p) m -> p c m", p=128),
                    )
                else:
                    eng.dma_start(
                        out=w_t_[:, :, sl],
                        in_=w_d_.rearrange("(c p) m -> p c m", p=128)[:, :, sl],
                    )

            def xdma(eng, e, j):
                eng.dma_start(
                    out=xt[e][:, ts(j, 512)],
                    in_=xT_d[e * 128:(e + 1) * 128, ts(j, 512)],
                )

            xq = {0: nc.sync, 1: nc.sync, 2: nc.sync, 3: nc.gpsimd,
                  4: nc.gpsimd, 5: nc.scalar, 6: nc.scalar, 7: nc.scalar}
            nc.scalar.dma_start(out=cosx[:, 0:512], in_=cos_d[:, 0:512])
            nc.scalar.dma_start(out=sinx[:, 0:512], in_=sin_d[:, 0:512])
            wdma(nc.sync, wk_t, wk_d, slice(0, 128))
            wdma(nc.gpsimd, wq_t, wq_d, slice(0, 128))
            for e in range(ECH):
                xdma(xq[e], e, 0)
            nc.gpsimd.dma_start(
                out=wv_t[:],
                in_=wv_d.rearrange("(c p) m -> p c m", p=128),
            )
            wdma(nc.sync, wk_t, wk_d, slice(128, 256))
            wdma(nc.scalar, wq_t, wq_d, slice(128, 256))
            for j in range(1, SB):
                for e in range(ECH):
                    xdma(xq[e], e, j)
                nc.scalar.dma_start(
                    out=cosx[:, ts(j, 512)], in_=cos_d[:, ts(j, 512)]
                )
                nc.scalar.dma_start(
                    out=sinx[:, ts(j, 512)], in_=sin_d[:, ts(j, 512)]
                )
            nc.sync.dma_start(
                out=wo_t[:],
                in_=wo_d.rearrange("(c p) e -> p c e", p=128),
            )
            # ones columns of the v tiles (cols 64:128 per head)
            for t in range(SCH):
                nc.gpsimd.memset(
                    vt[t].rearrange("p (h c) -> p h c", c=2 * D)[:, :, D:2 * D],
                    1.0,
                )

            # ---------------- work-unit emitters ----------------
            def emit_qk_proj(c, kind, j):
                """Projection + rope of one 512-seq block of qT/kT slab c."""
                w_t_ = wq_t if kind == "q" else wk_t
                dest = qT[c] if kind == "q" else kT[c]
                ps = mmp.tile([128, 512], F32, tag="mm", name="mm")
                for e in range(ECH):
                    nc.tensor.matmul(
                        ps[:],
                        w_t_[:, e, c * 128:(c + 1) * 128],
                        xt[e][:, ts(j, 512)],
                        start=(e == 0),
                        stop=(e == ECH - 1),
                    )
                sw = bswp.tile([128, 512], F32, tag="swp", name="swp")
                nc.vector.stream_shuffle(out=sw[:], in_=ps[:], mask=swap_mask)
                nc.vector.tensor_mul(
                    out=sw[:], in0=sw[:], in1=sinx[:, ts(j, 512)]
                )
                nc.vector.tensor_mul(
                    out=dest[:, ts(j, 512)], in0=ps[:], in1=cosx[:, ts(j, 512)]
                )
                nc.vector.tensor_add(
                    out=dest[:, ts(j, 512)], in0=dest[:, ts(j, 512)], in1=sw[:]
                )

            def emit_v_proj(t):
                """V projection of one 128-seq chunk (all 4 heads)."""
                ps = mmp.tile([128, 512], F32, tag="mm", name="mm")
                for e in range(ECH):
                    nc.tensor.matmul(
                        ps[:, 0:HL],
                        xt[e][:, ts(t, 128)],
                        wv_t[:, e, :],
                        start=(e == 0),
                        stop=(e == ECH - 1),
                    )
                nc.vector.tensor_copy(
                    out=vt[t].rearrange("p (h c) -> p h c", c=2 * D)[:, :, 0:D],
                    in_=ps[:, 0:HL].rearrange("p (h c) -> p h c", c=D),
                )

            # attention unit state
            sc_of = {}
            pv_of = {}

            def emit_sc(u):
                c, j, tp, nt = u
                qs = [qT[c][0:64, :], qT[c][64:128, :]]
                ks = [kT[c][0:64, :], kT[c][64:128, :]]
                sc = [csc.tile([128, 1024], F32, tag="sc", name="sc")
                      for _ in range(2)]
                # head 0 on PE rows 0-63, head 1 on rows 64-127: the two
                # heads' score matmuls overlap in the array
                for half in range(2):
                    t = 2 * tp + half
                    off = max(t * 128 - j * 512, 0) if tp == nt // 2 - 1 else 0
                    for i in range(2):
                        nc.tensor.matmul(
                            sc[i][:, half * 512 + off:(half + 1) * 512],
                            ks[i][:, ts(t, 128)],
                            qs[i][:, j * 512 + off:(j + 1) * 512],
                            start=True,
                            stop=True,
                        )
                sc_of[u] = sc

            exm_of = {}

            def emit_exp(u):
                """ACT exp + gpsimd causal mask for one chunk-pair unit."""
                c, j, tp, nt = u
                sc = sc_of.pop(u)
                trimmed = (tp == nt // 2 - 1)  # offs 256/384: mostly masked
                exm = []
                for i in range(2):
                    ex = cexp.tile([128, 1024], F16, tag="ex", name="ex")
                    if trimmed:
                        nc.scalar.activation(
                            out=ex[:, 256:512], in_=sc[i][:, 256:512],
                            func=Exp, scale=0.125,
                        )
                        nc.scalar.activation(
                            out=ex[:, 896:1024], in_=sc[i][:, 896:1024],
                            func=Exp, scale=0.125,
                        )
                    else:
                        nc.scalar.activation(
                            out=ex[:], in_=sc[i][:], func=Exp, scale=0.125
                        )
                    exm.append(ex)
                for half in range(2):
                    t = 2 * tp + half
                    off = max(t * 128 - j * 512, 0) if trimmed else 0
                    if t >= nt - 4:  # diagonal chunk: causal mask
                        for i in range(2):
                            nc.gpsimd.affine_select(
                                out=exm[i][:, half * 512 + off:
                                           (half + 1) * 512],
                                in_=exm[i][:, half * 512 + off:
                                          (half + 1) * 512],
                                compare_op=is_ge,
                                fill=0.0,
                                base=(j * 512 - t * 128) + off,
                                channel_multiplier=-1,
                                pattern=[[1, 512 - off]],
                            )
                exm_of[u] = exm

            def emit_pv(u):
                """PE pv matmuls for one chunk-pair unit."""
                c, j, tp, nt = u
                hs = [2 * c, 2 * c + 1]
                if tp == 0:
                    pv_of[(c, j)] = [
                        cpv.tile([128, 512], F32, tag=f"pv{i}", name=f"pv{i}")
                        for i in range(2)
                    ]
                pv = pv_of[(c, j)]
                exm = exm_of.pop(u)
                trimmed = (tp == nt // 2 - 1)
                for half in range(2):
                    t = 2 * tp + half
                    off = max(t * 128 - j * 512, 0) if trimmed else 0
                    for i in range(2):
                        nc.tensor.matmul(
                            pv[i][:, off:512],
                            vt[t].rearrange(
                                "p (h c) -> p h c", c=2 * D)[:, hs[i], :],
                            exm[i][:, half * 512 + off:(half + 1) * 512],
                            start=(t == 0),
                            stop=(t == nt - 1),
                        )

            def emit_norm(c, j):
                """softmax-normalize block j of slab c into oT[c].

                reciprocal_approx_fast (custom-DVE) only works at partition
                base 0 on HW and cannot read PSUM, so both heads' replicated
                Z rows are first copied (plain DVE ops handle PSUM reads and
                partition-base crossing fine) into one SBUF tile.
                """
                pv = pv_of.pop((c, j))
                zt = crb.tile([128, 512], F32, tag="rbr", name="rbr")
                nc.vector.tensor_copy(out=zt[0:64, :], in_=pv[0][64:128, :])
                nc.vector.tensor_copy(out=zt[64:128, :], in_=pv[1][64:128, :])
                nc.vector.reciprocal_approx_fast(out=zt[:, :], in_=zt[:, :])
                nc.vector.tensor_mul(
                    out=oT[c][0:64, ts(j, 512)],
                    in0=pv[0][0:64, :], in1=zt[0:64, :],
                )
                nc.vector.tensor_mul(
                    out=oT[c][64:128, ts(j, 512)],
                    in0=pv[1][0:64, :], in1=zt[64:128, :],
                )

            def emit_out(j, ti, n):
                """output projection of seq chunk ti (block j), half n."""
                t = 4 * j + ti
                ps = mmp.tile([128, 512], F32, tag="mm", name="wops")
                for c in range(2):
                    nc.tensor.matmul(
                        ps[:],
                        oT[c][:, ts(t, 128)],
                        wo_t[:, c, ts(n, 512)],
                        start=(c == 0),
                        stop=(c == 1),
                    )
                ot = ev.tile([128, 512], F16, tag="out", name="oev")
                nc.vector.tensor_copy(out=ot[:], in_=ps[:])
                nc.sync.dma_start(
                    out=out_d[ts(t, 128), ts(n, 512)], in_=ot[:]
                )

            # ---------------- the fused schedule ----------------
            # attention rounds in (j, c) order. Two-deep software pipeline:
            # at step n the ACT runs exp(n-1) while the PE runs pv(n-2) and
            # then sc(n) (which waits on exp(n-1) freeing the score psum).
            # Proj work for round r+1 (filler[r+1]) is interleaved between
            # the steps of round r so it is fully emitted before round r+1's
            # first score matmul; out-proj of block j rides as filler[2j+3].
            rounds = [(c, j) for j in range(SB) for c in range(2)]

            filler = {r: [] for r in range(len(rounds) + 3)}
            filler[0] += [
                lambda: emit_qk_proj(0, "k", 0),
                lambda: emit_qk_proj(0, "q", 0),
                lambda: emit_v_proj(0),
                lambda: emit_v_proj(1),
                lambda: emit_v_proj(2),
                lambda: emit_v_proj(3),
            ]
            need = {
                1: [(1, "k", 0), (1, "q", 0)],
                2: [(0, "k", 1), (0, "q", 1), ("v", 4), ("v", 5),
                    ("v", 6), ("v", 7)],
                3: [(1, "k", 1), (1, "q", 1)],
                4: [(0, "k", 2), (0, "q", 2), ("v", 8), ("v", 9),
                    ("v", 10), ("v", 11)],
                5: [(1, "k", 2), (1, "q", 2)],
                6: [(0, "k", 3), (0, "q", 3), ("v", 12), ("v", 13),
                    ("v", 14), ("v", 15)],
                7: [(1, "k", 3), (1, "q", 3)],
            }
            for r, items in need.items():
                for it in items:
                    if it[0] == "v":
                        filler[r].append(lambda t=it[1]: emit_v_proj(t))
                    else:
                        filler[r].append(
                            lambda c=it[0], k=it[1], j=it[2]:
                            emit_qk_proj(c, k, j)
                        )
            for j in range(SB):
                r = 2 * j + 4
                for ti in range(4):
                    for n in range(2):
                        filler[r].append(
                            lambda j=j, ti=ti, n=n: emit_out(j, ti, n)
                        )

            all_units = []
            unit_round = []
            for r, (c, j) in enumerate(rounds):
                nt = 4 * (j + 1)
                us = [(c, j, tp, nt) for tp in range(nt // 2)]
                all_units += us
                unit_round += [r] * len(us)
            n_units_in_round = {
                r: unit_round.count(r) for r in range(len(rounds))
            }

            fill_iters = {r: iter(f) for r, f in filler.items()}

            def take_filler(r, k=1):
                it = fill_iters.get(r)
                if it is None:
                    return
                for _ in range(k):
                    f = next(it, None)
                    if f is None:
                        return
                    f()

            def pv_and_norm(u):
                emit_pv(u)
                c, j, tp, nt = u
                if tp == nt // 2 - 1:
                    emit_norm(c, j)

            take_filler(0, 99)
            N = len(all_units)
            for un in range(N + 2):
                if 1 <= un <= N:
                    emit_exp(all_units[un - 1])
                if 2 <= un <= N + 1:
                    pv_and_norm(all_units[un - 2])
                if un < N:
                    r = unit_round[un]
                    nf = len(filler.get(r + 1, []))
                    per = (nf + n_units_in_round[r] - 1) // n_units_in_round[r]
                    take_filler(r + 1, max(per, 1))
                    emit_sc(all_units[un])
            # drain any remaining filler (last block's out-proj)
            for r in range(len(rounds) + 3):
                take_filler(r, 99)
            # warmup sink (keeps the warm-up matmuls live; late so its DMA
            # issue never delays the input streams)
            nc.sync.dma_start(out=wrm_d, in_=wrs[:])

    nc.compile()
    return nc


def _rope_tables():
    iexp = np.arange(0, D, 2, dtype=np.float32) / np.float32(D)
    inv_freq = np.reciprocal(np.power(np.float32(ROPE_BASE), iexp))  # (32,) f32
    ang = np.arange(S, dtype=np.float32)[:, None] * inv_freq[None, :]  # (S, 32)
    cos = np.cos(ang).astype(np.float32)  # (S, 32)
    sin = np.sin(ang).astype(np.float32)
    cosx = np.empty((64, S), dtype=np.float32)
    sinx = np.empty((64, S), dtype=np.float32)
    cosx[0::2] = cos.T
    cosx[1::2] = cos.T
    sinx[0::2] = -sin.T
    sinx[1::2] = sin.T
    return (np.tile(cosx, (2, 1)).astype(np.float16),
            np.tile(sinx, (2, 1)).astype(np.float16))  # (128, S) each


def get_nc():
    global _built
    if _built is None:
        _built = _build_nc()
    return _built


def make_in_maps(x, Wq, Wk, Wv, Wo):
    cosx, sinx = _rope_tables()
    in_maps = []
    for c in range(NCORES):
        b, g = c // 4, c % 4
        sl = slice(g * HL, (g + 1) * HL)
        in_maps.append({
            "xT": np.ascontiguousarray(x[b].T).astype(np.float16),
            "wq": np.ascontiguousarray(Wq[:, sl]).astype(np.float16),
            "wk": np.ascontiguousarray(Wk[:, sl]).astype(np.float16),
            "wv": np.ascontiguousarray(Wv[:, sl]).astype(np.float16),
            "wo": np.ascontiguousarray(Wo[sl, :]).astype(np.float16),
            "cosx": cosx,
            "sinx": sinx,
        })
    return in_maps


def gather(results):
    out = np.empty((B, S, E), dtype=np.float32)
    for b in range(B):
        acc = results[4 * b]["out"].astype(np.float32)
        for g in range(1, 4):
            acc = acc + results[4 * b + g]["out"].astype(np.float32)
        out[b] = acc
    return out


def kernel(x, Wq, Wk, Wv, Wo):
    from concourse.bass_utils import run_bass_kernel_spmd

    nc = get_nc()
    in_maps = make_in_maps(
        np.asarray(x), np.asarray(Wq), np.asarray(Wk), np.asarray(Wv), np.asarray(Wo)
    )
    res = run_bass_kernel_spmd(nc, in_maps, list(range(NCORES)))
    return gather(res.results)


# revision 16
# speedup vs baseline: 1.2003x; 1.2003x over previous
"""Multi-head causal attention with RoPE on 8 Trainium2 NeuronCores.

Sharding: tensor-parallel over heads x data-parallel over batch.
Core c handles batch b = c//4 and heads [4*(c%4), 4*(c%4)+4) (Hl=256 of Hd=1024).
Each core computes q/k/v projections for its head slice (column-split Wq/Wk/Wv),
RoPE, causal softmax attention, and a partial output projection (row-split Wo).
The host sums the 4 partial fp16 outputs per batch (the "all-reduce").

Fully fused single pipeline (v2): projections, rope, attention
(scores/exp/mask/pv), normalization and the output projection are emitted as
one interleaved instruction stream so the tensor engine never idles long
enough to re-throttle (HAM) and the ACT exp stream starts within a few us.

Device layouts (per core, S=2048, E=1024, Hl=256, D=64):
  xT   [E, S]    x transposed (host-side) so E rides the partition dim
  qT/kT slabs [128, S] x2: partitions = 2 heads x 64 dims, free = seq
  v    16 tiles [128, 512]: partitions = seq chunk, free = 4 heads x
       (64 dims + 64 ones cols) -> PV matmuls have M=128 and produce
       O on psum rows 0-63 and Z replicated on rows 64-127, so softmax
       normalization is two cross-partition-base reciprocal_approx_fast +
       two tensor_muls (no DRAM broadcast roundtrip).
  scores computed transposed (keys on partitions), exp on ACT (scale=0.125),
  causal masking via gpsimd affine_select on the diagonal chunks.

All matmul operands are fp16 (fp32 PSUM accumulation). A short burst of
dummy warm-up matmuls runs while the first input DMAs stream so the PE HAM
clock-gate is already released when real work arrives.
"""
import sys

sys.path.insert(0, "/opt/trn_rl_repo")
import numpy as np  # noqa: E402

N_HEADS = 16
B, S, E, HD = 2, 2048, 1024, 1024
D = HD // N_HEADS  # 64
HPC = 4            # heads per core
HL = HPC * D       # 256
NCORES = 8
ROPE_BASE = 10000.0

_built = None


def _build_nc():
    import concourse.bass as bass
    import concourse.tile as tile
    from concourse import bacc, mybir

    F32 = mybir.dt.float32
    F16 = mybir.dt.float16
    Exp = mybir.ActivationFunctionType.Exp
    is_ge = mybir.AluOpType.is_ge
    ts = bass.ts

    nc = bacc.Bacc("TRN2", target_bir_lowering=False, debug=False)
    xT_d = nc.dram_tensor("xT", [E, S], F16, kind="ExternalInput").ap()
    wq_d = nc.dram_tensor("wq", [E, HL], F16, kind="ExternalInput").ap()
    wk_d = nc.dram_tensor("wk", [E, HL], F16, kind="ExternalInput").ap()
    wv_d = nc.dram_tensor("wv", [E, HL], F16, kind="ExternalInput").ap()
    wo_d = nc.dram_tensor("wo", [HL, E], F16, kind="ExternalInput").ap()
    cos_d = nc.dram_tensor("cosx", [128, S], F16, kind="ExternalInput").ap()
    sin_d = nc.dram_tensor("sinx", [128, S], F16, kind="ExternalInput").ap()
    out_d = nc.dram_tensor("out", [S, E], F16, kind="ExternalOutput").ap()
    wrm_d = nc.dram_tensor("wrm", [1, 8], F32).ap()  # warmup sink

    ECH = E // 128   # 8 e-chunks
    SCH = S // 128   # 16 seq chunks
    SB = S // 512    # 4 seq blocks
    swap_mask = []
    for i in range(16):
        swap_mask += [2 * i + 1, 2 * i]

    with tile.TileContext(nc) as tc:
        with (
            tc.tile_pool(name="persist", bufs=1) as pp,
            tc.tile_pool(name="bswp", bufs=2) as bswp,
            tc.tile_pool(name="cexp", bufs=6) as cexp,
            tc.tile_pool(name="crb", bufs=2) as crb,
            tc.tile_pool(name="evict", bufs=4) as ev,
            tc.tile_pool(name="mm", bufs=2, space="PSUM") as mmp,
            tc.tile_pool(name="csc", bufs=2, space="PSUM") as csc,
            tc.tile_pool(name="cpv", bufs=1, space="PSUM") as cpv,
        ):
            # ---------------- persistent tiles ----------------
            qT = [pp.tile([128, S], F16, tag=f"qT{c}", name=f"qT{c}") for c in range(2)]
            kT = [pp.tile([128, S], F16, tag=f"kT{c}", name=f"kT{c}") for c in range(2)]
            vt = [pp.tile([128, HPC * 2 * D], F16, tag=f"v{t}", name=f"v{t}")
                  for t in range(SCH)]
            oT = [pp.tile([128, S], F16, tag=f"oT{c}", name=f"oT{c}") for c in range(2)]
            cosx = pp.tile([128, S], F16, tag="cosx", name="cosx")
            sinx = pp.tile([128, S], F16, tag="sinx", name="sinx")
            wo_t = pp.tile([128, 2, E], F16, tag="wo", name="wo")
            wq_t = pp.tile([128, ECH, HL], F16, tag="wq", name="wq")
            wk_t = pp.tile([128, ECH, HL], F16, tag="wk", name="wk")
            wv_t = pp.tile([128, ECH, HL], F16, tag="wv", name="wv")
            xt = [pp.tile([128, S], F16, tag=f"x{e}", name=f"x{e}")
                  for e in range(ECH)]
            wrm = pp.tile([128, 512], F16, tag="wrm", name="wrm")
            wrs = pp.tile([1, 8], F32, tag="wrs", name="wrs")

            # ---------------- PE warm-up (HAM release) ----------------
            # first gpsimd op so the warm-up matmuls start within ~1us
            nc.gpsimd.memset(wrm[:], 0.0)
            wps = mmp.tile([128, 512], F32, tag="mm", name="wps")
            for _ in range(12):
                nc.tensor.matmul(
                    wps[:], wrm[:, 0:128], wrm[:], start=True, stop=True
                )
            nc.vector.tensor_copy(out=wrs[:], in_=wps[0:1, 0:8])

            # ---------------- input DMAs ----------------
            # j0-critical data first, spread across four engine DMA queues
            # so the first projection can start ~4-5us in.
            def wdma(eng, w_t_, w_d_, sl=None):
                if sl is None:
                    eng.dma_start(
                        out=w_t_[:],
                        in_=w_d_.rearrange("(c p) m -> p c m", p=128),
                    )
                else:
                    eng.dma_start(
                        out=w_t_[:, :, sl],
                        in_=w_d_.rearrange("(c p) m -> p c m", p=128)[:, :, sl],
                    )

            def xdma(eng, e, j):
                eng.dma_start(
                    out=xt[e][:, ts(j, 512)],
                    in_=xT_d[e * 128:(e + 1) * 128, ts(j, 512)],
                )

            # j0-critical pieces split across the three issue-capable
            # queues; scalar (ACT) and gpsimd get ONLY early work - a busy
            # DMA ring blocks its issuing engine, and ACT/gpsimd have
            # critical exp/mask work from ~12us on. Everything else rides
            # the sync queue (sync has no other role).
            nc.scalar.dma_start(out=cosx[:, 0:512], in_=cos_d[:, 0:512])
            nc.scalar.dma_start(out=sinx[:, 0:512], in_=sin_d[:, 0:512])
            wdma(nc.sync, wk_t, wk_d, slice(0, 128))
            wdma(nc.gpsimd, wq_t, wq_d, slice(0, 128))
            xq = {0: nc.sync, 1: nc.sync, 2: nc.sync, 3: nc.gpsimd,
                  4: nc.gpsimd, 5: nc.gpsimd, 6: nc.scalar, 7: nc.scalar}
            for e in range(ECH):
                xdma(xq[e], e, 0)
            nc.sync.dma_start(
                out=wv_t[:],
                in_=wv_d.rearrange("(c p) m -> p c m", p=128),
            )
            wdma(nc.sync, wk_t, wk_d, slice(128, 256))
            wdma(nc.sync, wq_t, wq_d, slice(128, 256))
            for j in range(1, SB):
                for e in range(ECH):
                    xdma(nc.sync, e, j)
                nc.sync.dma_start(
                    out=cosx[:, ts(j, 512)], in_=cos_d[:, ts(j, 512)]
                )
                nc.sync.dma_start(
                    out=sinx[:, ts(j, 512)], in_=sin_d[:, ts(j, 512)]
                )
            nc.sync.dma_start(
                out=wo_t[:],
                in_=wo_d.rearrange("(c p) e -> p c e", p=128),
            )
            # ones columns of the v tiles (cols 64:128 per head)
            for t in range(SCH):
                nc.gpsimd.memset(
                    vt[t].rearrange("p (h c) -> p h c", c=2 * D)[:, :, D:2 * D],
                    1.0,
                )

            # ---------------- work-unit emitters ----------------
            def emit_qk_proj(c, kind, j):
                """Projection + rope of one 512-seq block of qT/kT slab c."""
                w_t_ = wq_t if kind == "q" else wk_t
                dest = qT[c] if kind == "q" else kT[c]
                ps = mmp.tile([128, 512], F32, tag="mm", name="mm")
                for e in range(ECH):
                    nc.tensor.matmul(
                        ps[:],
                        w_t_[:, e, c * 128:(c + 1) * 128],
                        xt[e][:, ts(j, 512)],
                        start=(e == 0),
                        stop=(e == ECH - 1),
                    )
                sw = bswp.tile([128, 512], F32, tag="swp", name="swp")
                nc.vector.stream_shuffle(out=sw[:], in_=ps[:], mask=swap_mask)
                nc.vector.tensor_mul(
                    out=sw[:], in0=sw[:], in1=sinx[:, ts(j, 512)]
                )
                nc.vector.tensor_mul(
                    out=dest[:, ts(j, 512)], in0=ps[:], in1=cosx[:, ts(j, 512)]
                )
                nc.vector.tensor_add(
                    out=dest[:, ts(j, 512)], in0=dest[:, ts(j, 512)], in1=sw[:]
                )

            def emit_v_proj(t):
                """V projection of one 128-seq chunk (all 4 heads)."""
                ps = mmp.tile([128, 512], F32, tag="mm", name="mm")
                for e in range(ECH):
                    nc.tensor.matmul(
                        ps[:, 0:HL],
                        xt[e][:, ts(t, 128)],
                        wv_t[:, e, :],
                        start=(e == 0),
                        stop=(e == ECH - 1),
                    )
                nc.vector.tensor_copy(
                    out=vt[t].rearrange("p (h c) -> p h c", c=2 * D)[:, :, 0:D],
                    in_=ps[:, 0:HL].rearrange("p (h c) -> p h c", c=D),
                )

            # attention unit state
            sc_of = {}
            pv_of = {}

            def emit_sc(u):
                c, j, tp, nt = u
                qs = [qT[c][0:64, :], qT[c][64:128, :]]
                ks = [kT[c][0:64, :], kT[c][64:128, :]]
                sc = [csc.tile([128, 1024], F32, tag="sc", name="sc")
                      for _ in range(2)]
                # head 0 on PE rows 0-63, head 1 on rows 64-127: the two
                # heads' score matmuls overlap in the array
                for half in range(2):
                    t = 2 * tp + half
                    off = max(t * 128 - j * 512, 0) if tp == nt // 2 - 1 else 0
                    for i in range(2):
                        nc.tensor.matmul(
                            sc[i][:, half * 512 + off:(half + 1) * 512],
                            ks[i][:, ts(t, 128)],
                            qs[i][:, j * 512 + off:(j + 1) * 512],
                            start=True,
                            stop=True,
                            tile_position=(64 * i, 0),
                        )
                sc_of[u] = sc

            exm_of = {}

            def emit_exp(u):
                """ACT exp + gpsimd causal mask for one chunk-pair unit."""
                c, j, tp, nt = u
                sc = sc_of.pop(u)
                trimmed = (tp == nt // 2 - 1)  # offs 256/384: mostly masked
                exm = []
                for i in range(2):
                    ex = cexp.tile([128, 1024], F16, tag="ex", name="ex")
                    if trimmed:
                        nc.scalar.activation(
                            out=ex[:, 256:512], in_=sc[i][:, 256:512],
                            func=Exp, scale=0.125,
                        )
                        nc.scalar.activation(
                            out=ex[:, 896:1024], in_=sc[i][:, 896:1024],
                            func=Exp, scale=0.125,
                        )
                    else:
                        nc.scalar.activation(
                            out=ex[:], in_=sc[i][:], func=Exp, scale=0.125
                        )
                    exm.append(ex)
                for half in range(2):
                    t = 2 * tp + half
                    off = max(t * 128 - j * 512, 0) if trimmed else 0
                    if t >= nt - 4:  # diagonal chunk: causal mask
                        for i in range(2):
                            nc.gpsimd.affine_select(
                                out=exm[i][:, half * 512 + off:
                                           (half + 1) * 512],
                                in_=exm[i][:, half * 512 + off:
                                          (half + 1) * 512],
                                compare_op=is_ge,
                                fill=0.0,
                                base=(j * 512 - t * 128) + off,
                                channel_multiplier=-1,
                                pattern=[[1, 512 - off]],
                            )
                exm_of[u] = exm

            def emit_pv(u):
                """PE pv matmuls for one chunk-pair unit."""
                c, j, tp, nt = u
                hs = [2 * c, 2 * c + 1]
                if tp == 0:
                    pv_of[(c, j)] = [
                        cpv.tile([128, 512], F32, tag=f"pv{i}", name=f"pv{i}")
                        for i in range(2)
                    ]
                pv = pv_of[(c, j)]
                exm = exm_of.pop(u)
                trimmed = (tp == nt // 2 - 1)
                for half in range(2):
                    t = 2 * tp + half
                    off = max(t * 128 - j * 512, 0) if trimmed else 0
                    for i in range(2):
                        nc.tensor.matmul(
                            pv[i][:, off:512],
                            vt[t].rearrange(
                                "p (h c) -> p h c", c=2 * D)[:, hs[i], :],
                            exm[i][:, half * 512 + off:(half + 1) * 512],
                            start=(t == 0),
                            stop=(t == nt - 1),
                        )

            def emit_norm(c, j):
                """softmax-normalize block j of slab c into oT[c].

                reciprocal_approx_fast (custom-DVE) only works at partition
                base 0 on HW and cannot read PSUM, so both heads' replicated
                Z rows are first copied (plain DVE ops handle PSUM reads and
                partition-base crossing fine) into one SBUF tile.
                """
                pv = pv_of.pop((c, j))
                zt = crb.tile([128, 512], F32, tag="rbr", name="rbr")
                nc.vector.tensor_copy(out=zt[0:64, :], in_=pv[0][64:128, :])
                nc.vector.tensor_copy(out=zt[64:128, :], in_=pv[1][64:128, :])
                nc.vector.reciprocal_approx_fast(out=zt[:, :], in_=zt[:, :])
                nc.vector.tensor_mul(
                    out=oT[c][0:64, ts(j, 512)],
                    in0=pv[0][0:64, :], in1=zt[0:64, :],
                )
                nc.vector.tensor_mul(
                    out=oT[c][64:128, ts(j, 512)],
                    in0=pv[1][0:64, :], in1=zt[64:128, :],
                )

            def emit_out(j, ti, n):
                """output projection of seq chunk ti (block j), half n."""
                t = 4 * j + ti
                ps = mmp.tile([128, 512], F32, tag="mm", name="wops")
                for c in range(2):
                    nc.tensor.matmul(
                        ps[:],
                        oT[c][:, ts(t, 128)],
                        wo_t[:, c, ts(n, 512)],
                        start=(c == 0),
                        stop=(c == 1),
                    )
                ot = ev.tile([128, 512], F16, tag="out", name="oev")
                nc.vector.tensor_copy(out=ot[:], in_=ps[:])
                nc.sync.dma_start(
                    out=out_d[ts(t, 128), ts(n, 512)], in_=ot[:]
                )

            # ---------------- the fused schedule ----------------
            # attention rounds in (j, c) order. Two-deep software pipeline:
            # at step n the ACT runs exp(n-1) while the PE runs pv(n-2) and
            # then sc(n) (which waits on exp(n-1) freeing the score psum).
            # Proj work for round r+1 (filler[r+1]) is interleaved between
            # the steps of round r so it is fully emitted before round r+1's
            # first score matmul; out-proj of block j rides as filler[2j+3].
            rounds = [(c, j) for j in range(SB) for c in range(2)]

            filler = {r: [] for r in range(len(rounds) + 3)}
            filler[0] += [
                lambda: emit_qk_proj(0, "k", 0),
                lambda: emit_qk_proj(0, "q", 0),
                lambda: emit_v_proj(0),
                lambda: emit_v_proj(1),
                lambda: emit_v_proj(2),
                lambda: emit_v_proj(3),
            ]
            need = {
                1: [(1, "k", 0), (1, "q", 0)],
                2: [(0, "k", 1), (0, "q", 1), ("v", 4), ("v", 5),
                    ("v", 6), ("v", 7)],
                3: [(1, "k", 1), (1, "q", 1)],
                4: [(0, "k", 2), (0, "q", 2), ("v", 8), ("v", 9),
                    ("v", 10), ("v", 11)],
                5: [(1, "k", 2), (1, "q", 2)],
                6: [(0, "k", 3), (0, "q", 3), ("v", 12), ("v", 13),
                    ("v", 14), ("v", 15)],
                7: [(1, "k", 3), (1, "q", 3)],
            }
            for r, items in need.items():
                for it in items:
                    if it[0] == "v":
                        filler[r].append(lambda t=it[1]: emit_v_proj(t))
                    else:
                        filler[r].append(
                            lambda c=it[0], k=it[1], j=it[2]:
                            emit_qk_proj(c, k, j)
                        )
            for j in range(SB):
                r = 2 * j + 4
                for ti in range(4):
                    for n in range(2):
                        filler[r].append(
                            lambda j=j, ti=ti, n=n: emit_out(j, ti, n)
                        )

            all_units = []
            unit_round = []
            for r, (c, j) in enumerate(rounds):
                nt = 4 * (j + 1)
                us = [(c, j, tp, nt) for tp in range(nt // 2)]
                all_units += us
                unit_round += [r] * len(us)
            n_units_in_round = {
                r: unit_round.count(r) for r in range(len(rounds))
            }

            fill_iters = {r: iter(f) for r, f in filler.items()}

            def take_filler(r, k=1):
                it = fill_iters.get(r)
                if it is None:
                    return
                for _ in range(k):
                    f = next(it, None)
                    if f is None:
                        return
                    f()

            def pv_and_norm(u):
                emit_pv(u)
                c, j, tp, nt = u
                if tp == nt // 2 - 1:
                    emit_norm(c, j)

            take_filler(0, 99)
            N = len(all_units)
            for un in range(N + 2):
                if 1 <= un <= N:
                    emit_exp(all_units[un - 1])
                if 2 <= un <= N + 1:
                    pv_and_norm(all_units[un - 2])
                if un < N:
                    r = unit_round[un]
                    nf = len(filler.get(r + 1, []))
                    per = (nf + n_units_in_round[r] - 1) // n_units_in_round[r]
                    take_filler(r + 1, max(per, 1))
                    emit_sc(all_units[un])
            # drain any remaining filler (last block's out-proj)
            for r in range(len(rounds) + 3):
                take_filler(r, 99)
            # warmup sink (keeps the warm-up matmuls live; late so its DMA
            # issue never delays the input streams)
            nc.sync.dma_start(out=wrm_d, in_=wrs[:])

    nc.compile()
    return nc


def _rope_tables():
    iexp = np.arange(0, D, 2, dtype=np.float32) / np.float32(D)
    inv_freq = np.reciprocal(np.power(np.float32(ROPE_BASE), iexp))  # (32,) f32
    ang = np.arange(S, dtype=np.float32)[:, None] * inv_freq[None, :]  # (S, 32)
    cos = np.cos(ang).astype(np.float32)  # (S, 32)
    sin = np.sin(ang).astype(np.float32)
    cosx = np.empty((64, S), dtype=np.float32)
    sinx = np.empty((64, S), dtype=np.float32)
    cosx[0::2] = cos.T
    cosx[1::2] = cos.T
    sinx[0::2] = -sin.T
    sinx[1::2] = sin.T
    return (np.tile(cosx, (2, 1)).astype(np.float16),
            np.tile(sinx, (2, 1)).astype(np.float16))  # (128, S) each


def get_nc():
    global _built
    if _built is None:
        _built = _build_nc()
    return _built


def make_in_maps(x, Wq, Wk, Wv, Wo):
    cosx, sinx = _rope_tables()
    in_maps = []
    for c in range(NCORES):
        b, g = c // 4, c % 4
        sl = slice(g * HL, (g + 1) * HL)
        in_maps.append({
            "xT": np.ascontiguousarray(x[b].T).astype(np.float16),
            "wq": np.ascontiguousarray(Wq[:, sl]).astype(np.float16),
            "wk": np.ascontiguousarray(Wk[:, sl]).astype(np.float16),
            "wv": np.ascontiguousarray(Wv[:, sl]).astype(np.float16),
            "wo": np.ascontiguousarray(Wo[sl, :]).astype(np.float16),
            "cosx": cosx,
            "sinx": sinx,
        })
    return in_maps


def gather(results):
    out = np.empty((B, S, E), dtype=np.float32)
    for b in range(B):
        acc = results[4 * b]["out"].astype(np.float32)
        for g in range(1, 4):
            acc = acc + results[4 * b + g]["out"].astype(np.float32)
        out[b] = acc
    return out


def kernel(x, Wq, Wk, Wv, Wo):
    from concourse.bass_utils import run_bass_kernel_spmd

    nc = get_nc()
    in_maps = make_in_maps(
        np.asarray(x), np.asarray(Wq), np.asarray(Wk), np.asarray(Wv), np.asarray(Wo)
    )
    res = run_bass_kernel_spmd(nc, in_maps, list(range(NCORES)))
    return gather(res.results)
